# revision 73
# baseline (speedup 1.0000x reference)
"""Multi-head attention TRN2 Bass kernel, head-sharded across 8 NeuronCores.

Problem: S=2048, E=1024, H=16 heads, dk=dv=64, fp32.
    Q = x @ Wq.T ; K = x @ Wk.T ; V = x @ Wv.T   (per-head slices)
    A_h = softmax(Q_h K_h^T / 8) V_h
    out = concat_h(A_h) @ Wo.T
Sharding: tensor-parallel over heads; core i owns heads (2i, 2i+1) and a
128-column slice of Wo. The 8 partial [2048,1024] outputs are summed on host.

Engine budget per core (the design drivers):
  - ACT exp is 2*S*S = 8.4M elems at 1 elem/lane/cycle @1.2GHz -> ~73us busy
    minimum (64 instrs of [128,1024]). It must start as early as possible and
    never starve; the steady state is ACT-bound (exp 1.11us/chunk vs ~0.86us
    of PE work), so emission order protects the exp chain above all.
  - Scores use zero-padded per-head K^T (K=128 full-array mode everywhere).
    Row-tiled tile_position scores were tried and dropped: mixing 64-row and
    128-row tile modes switches the PE array mode twice per chunk, and the
    steady state is ACT-bound anyway.
  - V is computed weight-stationary as V^T (8 LDWEIGHTS instead of 128) and
    DMA-transposed through the XBAR into the [sk, dv] layout AV needs, 4
    chunk-slots ahead of first use.  NOTE: dma_start_transpose needs a
    contiguous destination (gapped APs silently corrupt on HW), hence the
    vstage staging tile; and custom DVE ops (reciprocal_approx_fast) need
    partition-0 inputs on HW, hence the denominator row copy.

Emission order = per-engine queue order, so the loop is software-pipelined:
  x arrives in per-sequence-block pieces; K/Q/V projections of block t+1 are
  interleaved into block 0's attention chunks (one of K/Q/V per chunk slot);
  scores(c+1) is emitted before AV(c) so the PE streams scores while ACT
  does exp(c); outproj of block b-1 is emitted inside block b's chunk loop.

Softmax normalization rides the AV matmul as ones-columns in the stationary
operand (rows 64/65 of the [66,512] PSUM accumulator collect the exp row
sums); normalization is a reciprocal+broadcast multiply on DVE/GPSIMD.

All matmul operands bf16 (fp32 PSUM accumulation; ~0.5% rel err).
"""

import numpy as np
import ml_dtypes

import concourse.mybir as mybir
import concourse.tile as tile
from concourse import bacc
from concourse.bass_utils import run_bass_kernel_spmd

S, E, H, DK, DV = 2048, 1024, 16, 64, 64
NCORES = 8
HPC = H // NCORES          # heads per core = 2
CSL = HPC * DV             # concat-dim columns per core = 128
P = 128
NE = E // P                # 8 contraction chunks for projections
SQB = 512                  # sequence block (PSUM-bank-limited matmul width)
NSQB = S // SQB            # 4
NCH = S // P               # 16 sk chunks of 128
F32 = mybir.dt.float32
BF16 = mybir.dt.bfloat16
SCALE = 1.0 / np.sqrt(DK).astype(np.float32)  # 1/8

EXP = mybir.ActivationFunctionType.Exp
MULT = mybir.AluOpType.mult

_cache = {}
last_results = None  # BassKernelResults of the most recent run (for test.py)
TRACE = False
DEBUG = False


def _build_nc():
    nc = bacc.Bacc("TRN2", target_bir_lowering=False, debug=False)

    # host pre-arranges everything partition-major (and bf16) for fast DMA
    xT = nc.dram_tensor("xT", [P, NE, S], BF16, kind="ExternalInput")
    wqT = nc.dram_tensor("wqT", [P, NE, CSL], BF16, kind="ExternalInput")
    wkT = nc.dram_tensor("wkT", [P, NE, CSL], BF16, kind="ExternalInput")
    wvT = nc.dram_tensor("wvT", [P, NE, CSL], BF16, kind="ExternalInput")
    woT = nc.dram_tensor("woT", [CSL, E], BF16, kind="ExternalInput")
    y = nc.dram_tensor("y", [S, E], BF16, kind="ExternalOutput")

    dbg = {}
    if DEBUG:
        for nm, shp, dt in (
            ("dbg_q", [P, S], BF16), ("dbg_k", [P, S], BF16),
            ("dbg_va", [P, NCH, DV + 2], BF16), ("dbg_es", [P, 2 * SQB], BF16),
            ("dbg_at", [DV, SQB], F32), ("dbg_den", [1, SQB], F32),
            ("dbg_rsr", [1, SQB], F32),
            ("dbg_bc", [DV, SQB], F32), ("dbg_a1t", [P, SQB], BF16),
        ):
            dbg[nm] = nc.dram_tensor(nm, shp, dt, kind="ExternalOutput")

    xT_r = xT.ap()
    w_r = {"q": wqT.ap(), "k": wkT.ap(), "v": wvT.ap()}
    y_ap = y.ap()

    with tile.TileContext(nc) as tc:
        with tc.tile_pool(name="persist", bufs=1) as persist, \
             tc.tile_pool(name="xw", bufs=1) as xw, \
             tc.tile_pool(name="proj_ps", bufs=2, space="PSUM") as proj_ps, \
             tc.tile_pool(name="sc_ps", bufs=2, space="PSUM") as sc_ps, \
             tc.tile_pool(name="at_ps", bufs=2, space="PSUM") as at_ps, \
             tc.tile_pool(name="est", bufs=8) as est_pool, \
             tc.tile_pool(name="a1t", bufs=2) as a1t_pool, \
             tc.tile_pool(name="small", bufs=8) as small, \
             tc.tile_pool(name="outp", bufs=4) as outp:

            # Persistent SBUF tensors. qt/kt: rows 0-63 head A (dk), 64-127
            # head B.  vtsb: V^T in the same layout.  vaug[h]: V chunks in
            # [sk, dv] + 2 ones columns (softmax denominator rows).
            qt = persist.tile([P, S], BF16)
            # zero-padded per-head K^T so scores stay K=128 full-array mode
            # (mixing 64-row tile_position scores with 128-row AV matmuls
            # mode-switches the PE twice per chunk; measured NaN on HW).
            kpad = [
                persist.tile([P, S], BF16, name=f"kpad{h}", tag=f"kpad{h}")
                for h in range(HPC)
            ]
            vtsb = persist.tile([P, S], BF16)
            vaug = [
                persist.tile([P, NCH, DV + 2], BF16, name=f"vaug{h}", tag=f"vaug{h}")
                for h in range(HPC)
            ]
            # contiguous XBAR-transpose landing zone: dma_start_transpose on a
            # gapped destination (vaug's 66-stride) is wrong on HW, so land
            # in a dense [P, NCH, DV] tile and gpsimd-copy into vaug.
            vstage = [
                persist.tile([P, NCH, DV], BF16, name=f"vstage{h}", tag=f"vstage{h}")
                for h in range(HPC)
            ]
            wosb = persist.tile([P, E], BF16)

            # warm-up scratch memset first on the DVE queue: the warm-up
            # matmuls are the very first PE work and gate everything behind
            # them on the in-order queue
            scr = persist.tile([P, SQB], BF16, name="scr", tag="scr")
            nc.vector.memset(scr[:], 0.0)
            for h in range(HPC):
                nc.gpsimd.memset(vaug[h][:, :, DV : DV + 2], 1.0)
            # big zero-fills on DVE (idle at startup; gpsimd carries x DMAs)
            nc.vector.memset(kpad[0][DK:P, :], 0.0)
            nc.vector.memset(kpad[1][0:DK, :], 0.0)

            # Startup DMA order is the critical path to the first exp:
            # wk first, then x block 0 split across sync+gpsimd queues,
            # wq/wv behind them; wosb is deferred into block 0 (first
            # needed by outproj at block 1).
            wsb = {}
            for m in ("k", "q", "v"):
                wsb[m] = xw.tile([P, NE, CSL], BF16, name=f"w{m}sb", tag=f"w{m}")
            xsb = xw.tile([P, NE, S], BF16)

            def emit_xdma(t, queues):
                tsl = slice(t * SQB, (t + 1) * SQB)
                for n in range(NE):
                    queues[n % len(queues)].dma_start(
                        xsb[:, n, tsl], xT_r[:, n, tsl]
                    )

            nc.sync.dma_start(wsb["k"][:], w_r["k"][:])
            nc.gpsimd.dma_start(wsb["q"][:], w_r["q"][:])
            nc.scalar.dma_start(wsb["v"][:], w_r["v"][:])
            emit_xdma(0, [nc.sync, nc.gpsimd, nc.scalar])
            emit_xdma(1, [nc.gpsimd])

            # PE warm-up: the HAM clock gate holds the PE at 1.2 GHz until
            # ~3.4us of sustained activity.  Scratch matmuls (no data deps
            # beyond a memset) run while the x DMAs land, so the real
            # projections start at 2.4 GHz instead of paying 2x cycles.
            # 6 cold MMs (~2.6us) end as x-t0 arrives; K-proj continues the
            # activity window, so HAM still flips to 8/8 without the
            # warm-ups delaying K on the in-order PE queue.
            def emit_warmup(n_mm):
                for _ in range(n_mm):
                    wps = proj_ps.tile([P, SQB], F32, tag="proj")
                    nc.tensor.matmul(
                        wps[:], lhsT=scr[:, 0:P], rhs=scr[:],
                        start=True, stop=True,
                    )

            emit_warmup(6)

            def emit_proj(m, t, dst):
                """One 512-col block of a projection, weight-stationary,
                accumulated over the 8 E-chunks; cast into dst (bf16).
                dst=None means K: split-cast into the two padded K^T tiles."""
                tsl = slice(t * SQB, (t + 1) * SQB)
                ps = proj_ps.tile([P, SQB], F32, tag="proj")
                for n in range(NE):
                    nc.tensor.matmul(
                        ps[:], lhsT=wsb[m][:, n, :], rhs=xsb[:, n, tsl],
                        start=(n == 0), stop=(n == NE - 1),
                    )
                if dst is None:
                    nc.vector.tensor_copy(kpad[0][0:DK, tsl], ps[0:DK, :])
                    nc.vector.tensor_copy(kpad[1][DK:P, tsl], ps[DK:P, :])
                else:
                    nc.vector.tensor_copy(dst[:, tsl], ps[:])

            def emit_vtrans(t):
                """XBAR-transpose V^T block t into vstage[h][:, 4t:4t+4, :].
                out[p, c, j] = vtsb[64h+j, 512t + 128c + p] = V_h[sk, j];
                then copy into the 66-stride vaug layout on gpsimd."""
                csl4 = slice(4 * t, 4 * t + 4)
                for h in range(HPC):
                    # always on sync: a DMA_TRANSPOSE on the scalar queue
                    # lands between exp instructions and stalls ACT for
                    # the whole transfer (measured 7us bubble).
                    nc.sync.dma_start_transpose(
                        vstage[h][:, csl4, :],
                        vtsb[64 * h : 64 * h + 64, t * SQB : (t + 1) * SQB],
                    )
                    nc.vector.tensor_copy(
                        vaug[h][:, csl4, 0:DV], vstage[h][:, csl4, :]
                    )

            # K/Q of block 0 up front — the minimum for the first exp.
            emit_proj("k", 0, None)
            emit_proj("q", 0, qt)

            def emit_scores(b, c):
                """Both heads' scores^T chunk; zero-padded K keeps the PE in
                a single 128x128 tile mode."""
                bsl = slice(b * SQB, (b + 1) * SQB)
                csl = slice(c * P, (c + 1) * P)
                sc = sc_ps.tile([P, 2 * SQB], F32, tag="sc")
                for h in range(HPC):
                    nc.tensor.matmul(
                        sc[:, h * SQB : (h + 1) * SQB],
                        lhsT=kpad[h][:, csl], rhs=qt[:, bsl],
                        start=True, stop=True,
                    )
                return sc

            def emit_outproj_j(b, a1t, j, act_casts=False):
                """One 128-row slice of the output projection + its y DMA.
                act_casts: split PSUM->SBUF casts across DVE and ACT — only
                for the last block, where ACT has no exp work left and the
                epilogue is the exposed tail."""
                rsl = slice(b * SQB + j * P, b * SQB + (j + 1) * P)
                osb = outp.tile([P, E], BF16, tag="osb")
                q = nc.sync if j % 2 else nc.gpsimd
                for e2 in range(E // SQB):
                    esl = slice(e2 * SQB, (e2 + 1) * SQB)
                    ops = proj_ps.tile([P, SQB], F32, tag="proj")
                    nc.tensor.matmul(
                        ops[:], lhsT=a1t[:, j * P : (j + 1) * P],
                        rhs=wosb[:, esl], start=True, stop=True,
                    )
                    if act_casts and e2 % 2:
                        nc.scalar.copy(osb[:, esl], ops[:])
                    else:
                        nc.vector.tensor_copy(osb[:, esl], ops[:])
                    if act_casts:
                        # tail block: ship each half as soon as it's cast
                        q.dma_start(y_ap[rsl, esl], osb[:, esl])
                if not act_casts:
                    q.dma_start(y_ap[rsl, :], osb[:])

            def emit_outproj(b, a1t):
                for j in range(NSQB):
                    emit_outproj_j(b, a1t, j, act_casts=(b == NSQB - 1))

            def emit_normalize(at, a1t, dump=False, last=False):
                """a1t rows = A^T * (1/rowsum).  Bulk-copy the PSUM
                accumulators to SBUF first so the at banks free up fast
                (next block's AV reuses them) — except for the last block,
                where nothing reuses the banks and the copies only lengthen
                the exposed tail; there the multiply reads PSUM directly.
                Head B first: its partition-shift DMA (lane-aligned ops
                can't cross partitions) overlaps head A's multiply.
                Denominator rows go to a partition-0 tile either way:
                custom DVE ops (reciprocal_approx_fast) mis-read
                partition-offset single-row inputs on HW."""
                atsb, dens = [], []
                for h in range(HPC):
                    if last:
                        atsb.append(at[h])
                    else:
                        a = small.tile([DV, SQB], F32, name=f"atsb{h}", tag="atsb")
                        nc.vector.tensor_copy(a[:], at[h][0:DV, :])
                        atsb.append(a)
                    d = small.tile([1, SQB], F32, name=f"den{h}", tag="den")
                    nc.vector.tensor_copy(d[:], at[h][DV : DV + 1, :])
                    dens.append(d)
                if dump:
                    nc.sync.dma_start(dbg["dbg_den"].ap(), dens[0][:])
                    if not last:
                        nc.sync.dma_start(dbg["dbg_at"].ap(), atsb[0][:])
                for h in (1, 0):
                    rsr = small.tile([1, SQB], F32, tag="rsr")
                    nc.vector.reciprocal_approx_fast(rsr[:], dens[h][:])
                    bc = small.tile([DV, SQB], F32, tag="bc")
                    nc.gpsimd.partition_broadcast(bc[:], rsr[:])
                    if dump and h == 0:
                        nc.sync.dma_start(dbg["dbg_rsr"].ap(), rsr[:])
                        nc.sync.dma_start(dbg["dbg_bc"].ap(), bc[:])
                    if h == 0:
                        nc.vector.tensor_tensor(
                            a1t[0:DV, :], atsb[h][0:DV, :], bc[:], MULT
                        )
                    else:
                        tb = small.tile([DV, SQB], BF16, tag="tb")
                        nc.vector.tensor_tensor(
                            tb[:], atsb[h][0:DV, :], bc[:], MULT
                        )
                        nc.gpsimd.dma_start(a1t[DV:P, :], tb[:])

            prev_a1t = None
            sc = None
            for b in range(NSQB):
                at = [
                    at_ps.tile([P, SQB], F32, name=f"at{h}", tag="at")
                    for h in range(HPC)
                ]
                a1t = a1t_pool.tile([P, SQB], BF16, tag="a1t")
                if sc is None:
                    sc = emit_scores(b, 0)
                def emit_vchunk_direct(cc):
                    """V chunk cc in [sk, dv] orientation, x-stationary —
                    LDWEIGHTS-heavy but transpose-free.  Used for chunks
                    0-7 only: their XBAR transposes would otherwise queue
                    behind ~4MB of x DMA (measured 9us pipeline stall)."""
                    vp = proj_ps.tile([P, SQB], F32, tag="proj")
                    csl = slice(cc * P, (cc + 1) * P)
                    for n in range(NE):
                        nc.tensor.matmul(
                            vp[:, 0:P], lhsT=xsb[:, n, csl], rhs=wsb["v"][:, n, :],
                            start=(n == 0), stop=(n == NE - 1),
                        )
                    nc.vector.tensor_copy(vaug[0][:, cc, 0:DV], vp[:, 0:DV])
                    nc.vector.tensor_copy(vaug[1][:, cc, 0:DV], vp[:, DV:P])

                if b == 0:
                    emit_vchunk_direct(0)
                    emit_vchunk_direct(1)
                def emit_av(cc, es_cc):
                    for h in range(HPC):
                        nc.tensor.matmul(
                            at[h][0 : DV + 2, :],
                            lhsT=vaug[h][:, cc, :],
                            rhs=es_cc[:, h * SQB : (h + 1) * SQB],
                            start=(cc == 0), stop=(cc == NCH - 1),
                        )

                es_q = []
                for c in range(NCH):
                    es = est_pool.tile([P, 2 * SQB], BF16, tag="est")
                    nc.scalar.activation(es[:], sc[:], EXP, scale=float(SCALE))
                    es_q.append(es)
                    if DEBUG and b == 0 and c == 0:
                        nc.sync.dma_start(dbg["dbg_es"].ap(), es[:])
                    # scores(c+1) goes on the PE queue immediately after the
                    # exp so the exp cadence never waits on interleaved
                    # work; AV(c) and the interleaves run behind it (the es
                    # pool depth absorbs the lag).
                    if c < NCH - 1:
                        sc = emit_scores(b, c + 1)
                    elif b < NSQB - 1:
                        # next block's first scores right behind the last
                        # exp: zero exp gap across the block boundary
                        sc = emit_scores(b + 1, 0)
                    # interleave non-ACT-critical PE work behind the exp:
                    if b == 0 and c < 10:
                        # K of block t lands 4 chunk-slots before block-0
                        # attention reaches its chunks.  V chunks 2-7 are
                        # computed direct (transpose-free) just-in-time;
                        # V of t2/t3 uses the cheap flipped+XBAR path whose
                        # transposes run after the x DMA stream drains.
                        t, r = c // 4 + 1, c % 4
                        if c == 0:
                            emit_xdma(2, [nc.gpsimd])
                        elif c == 2:
                            emit_xdma(3, [nc.gpsimd])
                        elif c == 3:
                            nc.gpsimd.dma_start(wosb[:], woT.ap())
                        if c < 6:
                            emit_vchunk_direct(c + 2)
                        if r == 0 and t >= 2:
                            emit_proj("v", t, vtsb)
                            emit_vtrans(t)
                        elif r == 1:
                            emit_proj("k", t, None)
                    if b < NSQB - 1 and c == 8:
                        # next block's Q projection in this block's slack
                        # (spreading it 2 MMs/slot over c=10..13 measured
                        # neutral-to-worse: the held psum tile serializes
                        # against the pool rotation)
                        emit_proj("q", b + 1, qt)
                    if b > 0 and 2 <= c < 6:
                        # spread the previous block's output projection
                        # 2 MMs per chunk slot so exp cadence stays smooth
                        emit_outproj_j(b - 1, prev_a1t, c - 2)
                    # AV runs 4 chunks behind: transpose-chain / interleave
                    # latency on vaug/es never blocks the scores->exp chain,
                    # and the previous block's normalize gets a ~4.4us
                    # window to release the at accumulator banks before
                    # this block's first AV claims them.
                    if c >= 4:
                        emit_av(c - 4, es_q[c - 4])
                for cc in range(NCH - 4, NCH):
                    emit_av(cc, es_q[cc])
                if b == NSQB - 1:
                    # keep the PE warm through the whole ~6us normalize
                    # window so the final output projection doesn't run at
                    # the throttled clock (HAM re-throttles after 3.4us idle)
                    emit_warmup(24)
                emit_normalize(at, a1t, dump=(DEBUG and b == 0),
                               last=(b == NSQB - 1))
                if DEBUG and b == 0:
                    nc.sync.dma_start(dbg["dbg_a1t"].ap(), a1t[:])
                prev_a1t = a1t
            emit_outproj(NSQB - 1, prev_a1t)

            if DEBUG:
                nc.sync.dma_start(dbg["dbg_q"].ap(), qt[:])
                nc.sync.dma_start(dbg["dbg_k"].ap(), kpad[0][:])
                nc.sync.dma_start(dbg["dbg_va"].ap(), vaug[0][:])

    nc.compile()
    return nc


def kernel(x, Wq, Wk, Wv, Wo):
    global last_results
    x = np.asarray(x, dtype=np.float32)
    Wq = np.asarray(Wq, dtype=np.float32)
    Wk = np.asarray(Wk, dtype=np.float32)
    Wv = np.asarray(Wv, dtype=np.float32)
    Wo = np.asarray(Wo, dtype=np.float32)

    if "nc" not in _cache:
        _cache["nc"] = _build_nc()
    nc = _cache["nc"]

    bf = ml_dtypes.bfloat16
    # [E, S] -> [P, NE, S] partition-major (chunk n, partition p = row n*P+p)
    xT = np.ascontiguousarray(
        x.T.reshape(NE, P, S).transpose(1, 0, 2).astype(bf)
    )
    WqT = np.ascontiguousarray(Wq.T)
    WkT = np.ascontiguousarray(Wk.T)
    WvT = np.ascontiguousarray(Wv.T)
    WoT = np.ascontiguousarray(Wo.T)

    in_maps = []
    for i in range(NCORES):
        sl = slice(i * CSL, (i + 1) * CSL)

        def wslice(WT):
            # [E, CSL] slice -> [P, NE, CSL] partition-major
            return np.ascontiguousarray(
                WT[:, sl].reshape(NE, P, CSL).transpose(1, 0, 2).astype(bf)
            )

        in_maps.append({
            "xT": xT,
            "wqT": wslice(WqT),
            "wkT": wslice(WkT),
            "wvT": wslice(WvT),
            "woT": np.ascontiguousarray(WoT[sl, :].astype(bf)),
        })

    last_results = run_bass_kernel_spmd(
        nc, in_maps, core_ids=list(range(NCORES)), trace=TRACE
    )
    out = np.zeros((S, E), dtype=np.float32)
    for r in last_results.results:
        out += r["y"].astype(np.float32)
    return out
